# revision 25
# baseline (speedup 1.0000x reference)
"""GPT-J attention (B=2, S=2048, D=4096, 16 heads x 256, partial RoPE 64) on 8 trn2 cores.

Tensor-parallel over heads: each core owns 2 heads (Wq/Wk/Wv column slices,
Wo row slice). Per-oc-column bf16 ReduceScatter sums the partial out-projections
and leaves each core a 512-token shard of [B*S, D]; host concatenates shards.

v2 kernel strategy (all matmul inputs bf16, PSUM accumulation fp32):
  - Phase A per b: QKV projections accumulate over ALL 32 d-chunks directly in
    PSUM (no SBUF partial-sum round trips). Five passes per 512-token tile
    (Q01, Q23, K01, K23, V) using <= 6 PSUM banks with evacuation (alternating
    ACT/DVE) pipelined behind the next pass. Weights and hsT streamed in
    host-pre-swizzled partition-contiguous blocks. RoPE folded in per tile.
  - Phase B: scores computed TRANSPOSED (ssT[k,q] = KT_chunk^T @ QT) so the
    exp output is directly P^T, feeding PV with V as stationary - no PE
    transposes, no P copies. Row sums via a ones-column matmul into PSUM
    (partitions 0/32/64/96 of 2 banks), then reciprocal + one small PE
    transpose gives recip in q-partition layout. Softmax normalization is
    deferred to the phase-C evacuation. Software-pipelined: scores(kc+1)
    emitted before PV(kc).
  - Phase C per b, oc-outer: y partials accumulate per (sg,h) in PSUM;
    h=0 evacuated by ACT (activation Copy with per-partition recip scale),
    h=1 by DVE fused mult-add. bf16 y_part written per oc block; after b=1
    finishes an oc block, a ReduceScatter(add) for just that [4096,512] block
    fires, overlapping the remaining compute.
"""

import os
import sys

import numpy as np

sys.path.insert(0, "/opt/trn_rl_repo")

# ---------------------------------------------------------------- constants
B = 2
S = 2048
D = 4096
NH = 16
HD = 256
ROT = 64
MAX_POS = 2048
N_CORES = 8
HPC = NH // N_CORES          # heads per core = 2
HDL = HPC * HD               # local head width = 512

SC = 512                     # token tile / q macro tile / moving width
NEG = -1.0e30


def _cfg_full():
    return dict(B=B, S=S, D=D, HPC=HPC, HD=HD, ROT=ROT)


# ---------------------------------------------------------------- bass build

def build_nc(cfg, use_collective=True, n_cores=N_CORES):
    import concourse.tile as tile
    from concourse import bacc, mybir

    fp32 = mybir.dt.float32
    bf16 = mybir.dt.bfloat16

    Bc, Sc, Dc, HPCc, HDc, ROTc = (
        cfg["B"], cfg["S"], cfg["D"], cfg["HPC"], cfg["HD"], cfg["ROT"])
    HDLc = HPCc * HDc                    # local head width (512)
    NHC = HDLc // 128                    # local hd chunks (4)
    NSC = Sc // SC                       # 512-token tiles per b (4)
    NDC = Dc // 128                      # d chunks (32)
    NG = NDC // 8                        # streamed weight groups (4)
    NOC = Dc // SC                       # out-proj column chunks (8)
    NKC = Sc // 128                      # k chunks per b (16)
    SHARD = (Bc * Sc) // n_cores if use_collective else Bc * Sc

    nc = bacc.Bacc(num_devices=n_cores)

    # inputs (per-core, host-pre-swizzled for contiguous DMA)
    hs_e = nc.declare_dram_parameter("hs_s", [Bc, NSC, 4, 128, NDC // 4, SC],
                                     bf16, isOutput=False)
    wq_e = nc.declare_dram_parameter("wq_s", [2, 2, 128, 16, HDLc // 2],
                                     bf16, isOutput=False)
    wk_e = nc.declare_dram_parameter("wk_s", [2, 2, 128, 16, HDLc // 2],
                                     bf16, isOutput=False)
    wv_e = nc.declare_dram_parameter("wv_s", [NG, 128, 8, HDLc],
                                     bf16, isOutput=False)
    wo_e = nc.declare_dram_parameter("wo_s", [NDC, 128, NDC, 128],
                                     bf16, isOutput=False)
    cos_e = nc.declare_dram_parameter("cosb", [Bc, ROTc, Sc], bf16, isOutput=False)
    sin_e = nc.declare_dram_parameter("sinb", [Bc, ROTc, Sc], bf16, isOutput=False)
    msk_e = nc.declare_dram_parameter("masksT", [128, 4, SC], bf16, isOutput=False)
    psw_e = nc.declare_dram_parameter("pswap", [128, ROTc], bf16, isOutput=False)
    one_e = nc.declare_dram_parameter("onesc", [128, 1], bf16, isOutput=False)
    onr_e = nc.declare_dram_parameter("onesr", [1, 128], fp32, isOutput=False)

    TPC = Sc // n_cores                  # tokens per core per batch (256)
    y_e = nc.declare_dram_parameter("y", [Dc, Bc * TPC], bf16, isOutput=True)
    a2a_in = [nc.dram_tensor(f"a2a_in{b}", [n_cores, 128, NHC, TPC], bf16)
              for b in range(Bc)]
    a2a_out = [nc.dram_tensor(f"a2a_out{b}", [n_cores, 128, NHC, TPC], bf16)
               for b in range(Bc)]

    def mm(ps, lhsT, rhs, start, stop):
        nc.tensor.matmul(ps, lhsT, rhs, start=start, stop=stop)

    with tile.TileContext(nc) as tc:
        with tc.tile_pool(name="const", bufs=1) as constp:
            masks = constp.tile([128, 4, SC], bf16)
            nc.sync.dma_start(masks[:], msk_e[:])
            pswap = constp.tile([128, ROTc], bf16)
            nc.sync.dma_start(pswap[:], psw_e[:])
            ones = constp.tile([128, 1], bf16)
            nc.sync.dma_start(ones[:], one_e[:])
            onesr = constp.tile([1, 128], fp32)
            nc.sync.dma_start(onesr[:], onr_e[:])

            with (
                tc.tile_pool(name="qkv", bufs=1) as qkvp,      # QT/KT/V one b
                tc.tile_pool(name="atn", bufs=1) as atnp,      # ATN one b
                tc.tile_pool(name="xn", bufs=1) as xnp,        # received x
                tc.tile_pool(name="bcs", bufs=2) as bcsb,
                tc.tile_pool(name="bcps", bufs=1, space="PSUM") as bcps,
            ):
                xn = [xnp.tile([128, NHC, Bc * TPC], bf16, tag=f"xn{s}",
                               name=f"xn{s}") for s in range(n_cores)]

                def emit_recv(b):
                    # receive pre-normalized xn columns for batch half b
                    bsl = slice(b * TPC, (b + 1) * TPC)
                    for s in range(n_cores):
                        nc.sync.dma_start(xn[s][:, :, bsl], a2a_out[b][s])

                for b in range(Bc):
                    # ============ phase A: QKV projection (PSUM-resident) ====
                    QT = [qkvp.tile([128, Sc], bf16, tag=f"QT{c}", name=f"QT{c}") for c in range(NHC)]
                    KT = [qkvp.tile([128, Sc], bf16, tag=f"KT{c}", name=f"KT{c}") for c in range(NHC)]
                    V = [qkvp.tile([128, HDLc], bf16, tag=f"V{k}", name=f"V{k}") for k in range(NKC)]

                    with (
                        tc.tile_pool(name="trig", bufs=1) as trigp,
                        tc.tile_pool(name="hst", bufs=5) as hp,
                        tc.tile_pool(name="wqk", bufs=3) as wqkp,
                        tc.tile_pool(name="wvs", bufs=3) as wvp,
                        tc.tile_pool(name="pjps", bufs=1, space="PSUM") as pjps,
                        tc.tile_pool(name="rops", bufs=2, space="PSUM") as ropsp,
                        tc.tile_pool(name="ropb", bufs=1) as ropbp,
                    ):
                        cosb = trigp.tile([ROTc, Sc], bf16, tag="cos")
                        sinb = trigp.tile([ROTc, Sc], bf16, tag="sin")
                        nc.sync.dma_start(cosb[:], cos_e[b])
                        nc.sync.dma_start(sinb[:], sin_e[b])
                        HQD = NDC // 4
                        for st in range(NSC):
                            ssl = slice(st * SC, (st + 1) * SC)
                            hq = []
                            for q4 in range(4):
                                hq.append(hp.tile([128, HQD, SC], bf16,
                                                  tag="hst", name="hst"))
                                nc.sync.dma_start(hq[q4][:], hs_e[b, st, q4])

                            def hst(dc, _hq=hq):
                                return _hq[dc // HQD][:, dc % HQD, :]

                            def rope(t, c):
                                # rotate rows 0:ROT of t[c] at columns ssl
                                sw = ropsp.tile([ROTc, SC], fp32, tag="rp")
                                mm(sw[:], pswap[:, :], t[c][:, ssl],
                                   start=True, stop=True)
                                t1 = ropbp.tile([ROTc, SC], bf16, tag="t1")
                                t2 = ropbp.tile([ROTc, SC], bf16, tag="t2")
                                nc.vector.tensor_tensor(
                                    t1[:], sw[:], sinb[:, ssl],
                                    op=mybir.AluOpType.mult)
                                nc.vector.tensor_tensor(
                                    t2[:], t[c][0:ROTc, ssl], cosb[:, ssl],
                                    op=mybir.AluOpType.mult)
                                nc.vector.tensor_add(
                                    t[c][0:ROTc, ssl], t1[:], t2[:])

                            # 4 Q/K passes (2 banks each) + 1 V pass (4 banks)
                            for pi, (we, dst, hf) in enumerate((
                                    (wq_e, QT, 0), (wq_e, QT, 1),
                                    (wk_e, KT, 0), (wk_e, KT, 1))):
                                bk = (pi % 2) * 2
                                t0 = pjps.tile([128, SC], fp32, tag=f"pj{bk}")
                                t1_ = pjps.tile([128, SC], fp32, tag=f"pj{bk + 1}")
                                for g in range(2):
                                    wa = wqkp.tile([128, 16, HDLc // 2], bf16,
                                                   tag="wa")
                                    nc.sync.dma_start(wa[:], we[hf, g])
                                    for j in range(16):
                                        dc = g * 16 + j
                                        st_, sp_ = (dc == 0), (dc == NDC - 1)
                                        mm(t0[:], wa[:, j, 0:128], hst(dc),
                                           start=st_, stop=sp_)
                                        mm(t1_[:], wa[:, j, 128:256], hst(dc),
                                           start=st_, stop=sp_)
                                for j, ps in enumerate((t0, t1_)):
                                    c = hf * 2 + j
                                    if pi % 2 == 0:
                                        nc.scalar.copy(dst[c][:, ssl], ps[:])
                                    else:
                                        nc.vector.tensor_copy(dst[c][:, ssl], ps[:])
                                if hf == 0:
                                    rope(dst, 0)
                                else:
                                    rope(dst, 2)

                            # V pass: stationary = hst chunks, moving = wv
                            psv = [pjps.tile([128, HDLc], fp32, tag=f"pj{ss}", name=f"psv{ss}")
                                   for ss in range(4)]
                            for g in range(NG):
                                wvt = wvp.tile([128, 8, HDLc], bf16, tag="wv")
                                nc.sync.dma_start(wvt[:], wv_e[g])
                                for j in range(8):
                                    dc = g * 8 + j
                                    st_, sp_ = (dc == 0), (dc == NDC - 1)
                                    for ss in range(4):
                                        mm(psv[ss][:],
                                           hst(dc)[:, ss * 128:(ss + 1) * 128],
                                           wvt[:, j, :], start=st_, stop=sp_)
                            for ss in range(4):
                                kcv = st * 4 + ss
                                if ss % 2 == 0:
                                    nc.scalar.copy(V[kcv][:], psv[ss][:])
                                else:
                                    nc.vector.tensor_copy(V[kcv][:], psv[ss][:])

                    # ============ phase B: attention (transposed scores) =====
                    if b == 1:
                        emit_recv(0)
                    ATN = atnp.tile([128, NHC, Sc], bf16, tag="ATN", name="ATN")
                    
                    with (
                        tc.tile_pool(name="ptb", bufs=1) as ptp,
                        tc.tile_pool(name="rsb", bufs=1) as rsbp,
                        tc.tile_pool(name="scps", bufs=1, space="PSUM") as scps,
                        tc.tile_pool(name="atps", bufs=1, space="PSUM") as atps,
                        tc.tile_pool(name="rsps", bufs=1, space="PSUM") as rsps,
                    ):
                        psRS = rsps.tile([128, SC], fp32, tag="rs0")
                        rrb = rsbp.tile([1, 8, SC], fp32, tag="rrec")

                        def emit_scores(h, qm, kc):
                            c0 = h * (HDc // 128)
                            qsl = slice(qm * SC, (qm + 1) * SC)
                            kcl = slice(kc * 128, (kc + 1) * 128)
                            ss = scps.tile([128, SC], fp32, tag=f"ss{kc % 3}",
                                           name=f"ss{kc % 3}")
                            mm(ss[:], KT[c0][:, kcl], QT[c0][:, qsl],
                               start=True, stop=False)
                            mm(ss[:], KT[c0 + 1][:, kcl], QT[c0 + 1][:, qsl],
                               start=False, stop=True)
                            return ss

                        for h in range(HPCc):
                            c0 = h * (HDc // 128)
                            for qm in range(NSC):
                                nkc = (qm + 1) * 4
                                at = [atps.tile([128, SC], fp32, tag=f"at{hh}", name=f"at{hh}")
                                      for hh in range(HDc // 128)]
                                ss_cur = emit_scores(h, qm, 0)
                                ss_nxt = (emit_scores(h, qm, 1)
                                          if nkc > 1 else None)
                                for kc in range(nkc):
                                    if kc // 4 == qm:   # diagonal macro tile
                                        nc.vector.tensor_add(
                                            ss_cur[:], ss_cur[:],
                                            masks[:, kc % 4, :])
                                    pt = ptp.tile([128, SC], bf16,
                                                  tag=f"pt{kc % 3}")
                                    nc.scalar.activation(
                                        pt[:], ss_cur[:],
                                        mybir.ActivationFunctionType.Exp,
                                        bias=0.0, scale=1.0 / 16.0)
                                    ss_fut = (emit_scores(h, qm, kc + 2)
                                              if kc + 2 < nkc else None)
                                    ss_cur, ss_nxt = ss_nxt, ss_fut
                                    st_, sp_ = (kc == 0), (kc == nkc - 1)
                                    for hh in range(HDc // 128):
                                        mm(at[hh][:],
                                           V[kc][:, h * HDc + hh * 128:
                                                 h * HDc + (hh + 1) * 128],
                                           pt[:], start=st_, stop=sp_)
                                    mm(psRS[0:1, :],
                                       ones[:], pt[:], start=st_, stop=sp_)
                                nc.vector.reciprocal(
                                    rrb[0:1, h * NSC + qm, :], psRS[0:1, :])
                                qsl = slice(qm * SC, (qm + 1) * SC)
                                for hh in range(HDc // 128):
                                    if hh == 0:
                                        nc.scalar.copy(ATN[:, c0 + hh, qsl],
                                                       at[hh][:])
                                    else:
                                        nc.vector.tensor_copy(
                                            ATN[:, c0 + hh, qsl], at[hh][:])
                                i2 = (h * NSC + qm) % 2
                                bc_ps = bcps.tile([128, SC], fp32,
                                                  tag=f"bc{i2}", name=f"bc{i2}")
                                mm(bc_ps[:], onesr[:],
                                   rrb[0:1, h * NSC + qm, :],
                                   start=True, stop=True)
                                bcs = bcsb.tile([128, SC], bf16, tag=f"bcs{i2}",
                                                name=f"bcs{i2}")
                                nc.vector.tensor_copy(bcs[:], bc_ps[:])
                                for hh in range(HDc // 128):
                                    nc.vector.tensor_tensor(
                                        ATN[:, c0 + hh, qsl],
                                        ATN[:, c0 + hh, qsl],
                                        bcs[:], op=mybir.AluOpType.mult)

                        # ship ATN token-slices + recips to owning cores
                        for r in range(n_cores):
                            nc.scalar.dma_start(
                                a2a_in[b][r],
                                ATN[:, :, r * TPC:(r + 1) * TPC])
                        if use_collective:
                            nc.gpsimd.collective_compute(
                                "AllToAll", mybir.AluOpType.bypass,
                                replica_groups=[list(range(n_cores))],
                                ins=[a2a_in[b][:]], outs=[a2a_out[b][:]])
                        else:
                            nc.sync.dma_start(a2a_out[b][:], a2a_in[b][:])


                # ===== phase C': local out-projection over owned tokens ==
                # two half-token passes: half 0 (b=0 columns) starts right
                # after B(b1) and hides the b=1 AllToAll; half 1 follows
                with (
                    tc.tile_pool(name="wo2", bufs=6) as wo2p,
                    tc.tile_pool(name="ysbT", bufs=2) as ysbtp,
                    tc.tile_pool(name="ypsT", bufs=2, space="PSUM") as ypstp,
                ):
                    for half in range(Bc):
                        if half == 1:
                            emit_recv(1)
                        hsl = slice(half * TPC, (half + 1) * TPC)
                        for dsub in range(NDC):
                            wo2 = wo2p.tile([128, NDC, 128], bf16, tag="wo2")
                            nc.sync.dma_start(wo2[:], wo_e[dsub])
                            yT = ypstp.tile([128, TPC], fp32,
                                            tag=f"yT{dsub % 2}",
                                            name=f"yT{dsub % 2}")
                            for cg in range(NDC):
                                s, c = cg // NHC, cg % NHC
                                mm(yT[:], wo2[:, cg, :], xn[s][:, c, hsl],
                                   start=(cg == 0), stop=(cg == NDC - 1))
                            ysbT = ysbtp.tile([128, TPC], bf16, tag="ysbT")
                            if dsub % 2 == 0:
                                nc.scalar.copy(ysbT[:], yT[:])
                            else:
                                nc.vector.tensor_copy(ysbT[:], yT[:])
                            nc.gpsimd.dma_start(
                                y_e[dsub * 128:(dsub + 1) * 128, hsl],
                                ysbT[:])

    nc.compile()
    return nc


# ---------------------------------------------------------------- host prep

def _sinusoidal_np(num_pos, dim):
    inv_freq = 1.0 / (10000.0 ** (np.arange(0, dim, 2, dtype=np.float32) / dim))
    t = np.arange(num_pos, dtype=np.float32)[:, None] * inv_freq[None, :]
    return np.cos(t).astype(np.float32), np.sin(t).astype(np.float32)


def _host_arrays(hs, Wq, Wk, Wv, Wo, position_ids, cfg, n_cores):
    """Build the shared + per-core input arrays (pre-swizzled, bf16)."""
    import ml_dtypes
    bf = ml_dtypes.bfloat16
    Bc, Sc, Dc, HPCc, HDc, ROTc = (
        cfg["B"], cfg["S"], cfg["D"], cfg["HPC"], cfg["HD"], cfg["ROT"])
    HDLc = HPCc * HDc
    NSCc, NDCc, NGc, NOCc, NHCc = Sc // SC, Dc // 128, Dc // 1024, Dc // SC, HDLc // 128

    # hs_s[b, st, hf, p, j, f] = hs[b, st*SC+f, (hf*16+j)*128+p]
    hs_s = np.ascontiguousarray(
        hs.reshape(Bc, NSCc, SC, 4, NDCc // 4, 128)
        .transpose(0, 1, 3, 5, 4, 2)).astype(bf)

    cos_t, sin_t = _sinusoidal_np(max(MAX_POS, Sc), ROTc)   # [P, ROT//2]
    pos = np.asarray(position_ids).astype(np.int64)         # [B, S]
    cosg = cos_t[pos]                                       # [B, S, 32]
    sing = sin_t[pos]
    cosb = np.repeat(cosg.transpose(0, 2, 1), 2, axis=1)    # [B, 64, S]
    sinb_r = np.repeat(sing.transpose(0, 2, 1), 2, axis=1)
    sgn = np.ones((ROTc, 1), np.float32)
    sgn[0::2] = -1.0
    sinb = np.ascontiguousarray(sinb_r * sgn).astype(bf)
    cosb = np.ascontiguousarray(cosb).astype(bf)

    # transposed causal masks for diagonal 512 macro tile: masksT[k, kc, q]
    masksT = np.zeros((128, 4, SC), np.float32)
    kk = np.arange(128)[:, None]
    qq = np.arange(SC)[None, :]
    for m in range(4):
        masksT[:, m, :] = np.where(m * 128 + kk <= qq, 0.0, NEG)
    masksT = masksT.astype(bf)

    pswap = np.zeros((128, ROTc), np.float32)
    for f in range(ROTc // 2):
        pswap[2 * f + 1, 2 * f] = 1.0
        pswap[2 * f, 2 * f + 1] = 1.0
    onesc = np.ones((128, 1), np.float32).astype(bf)
    onesr = np.ones((1, 128), np.float32)

    # wo2_s[dsub, p, cg, m] = Wo[dsub*128+m, cg*128+p] (full Wo, shared)
    wo2_s = np.ascontiguousarray(
        np.asarray(Wo).reshape(NDCc, 128, NDCc, 128)
        .transpose(0, 3, 2, 1)).astype(bf)

    shared = dict(hs_s=hs_s, cosb=cosb, sinb=sinb, masksT=masksT,
                  pswap=pswap.astype(bf), onesc=onesc, onesr=onesr,
                  wo_s=wo2_s)

    def _wswz_qk(w):   # [HDLc(rows of W slice), Dc] -> [2, NG, 128, 8, HDLc//2]
        # w here is the [HDLc, Dc] row-slice of the full weight; stationary
        # layout wq_s[hf, g, p, j, f] = w[hf*256+f, (g*8+j)*128+p]
        return np.ascontiguousarray(
            w.reshape(2, HDLc // 2, 2, 16, 128)
            .transpose(0, 2, 4, 3, 1)).astype(bf)

    def _wswz_v(w):    # -> [NG, 128, 8, HDLc];  wv_s[g,p,j,f] = w[f,(g*8+j)*128+p]
        return np.ascontiguousarray(
            w.reshape(HDLc, NGc, 8, 128).transpose(1, 3, 2, 0)).astype(bf)

    per_core = []
    for c in range(n_cores):
        csl = slice(c * HDLc, (c + 1) * HDLc)
        per_core.append(dict(
            wq_s=_wswz_qk(np.asarray(Wq)[csl, :]),
            wk_s=_wswz_qk(np.asarray(Wk)[csl, :]),
            wv_s=_wswz_v(np.asarray(Wv)[csl, :]),
            **shared,
        ))
    return per_core


def _numpy_reference(hidden_states, Wq, Wk, Wv, Wo, layer_past_k, layer_past_v,
                     attention_mask, position_ids, new_key_loc, new_value_loc,
                     valid_key_indices, valid_value_indices, bucket_size):
    """Slow but general fallback (mirrors reference.py in numpy fp32)."""
    hs = np.asarray(hidden_states, np.float32)
    Bc, Sc, Dc = hs.shape
    q = (hs @ np.asarray(Wq).T).reshape(Bc, Sc, NH, HD)
    k = (hs @ np.asarray(Wk).T).reshape(Bc, Sc, NH, HD)
    v = (hs @ np.asarray(Wv).T).reshape(Bc, Sc, NH, HD)

    cos_t, sin_t = _sinusoidal_np(MAX_POS, ROT)
    pos = np.asarray(position_ids).astype(np.int64)
    c_ = cos_t[pos][:, :, None, :]      # [B,S,1,32]
    s_ = sin_t[pos][:, :, None, :]

    def rot(x):
        xr = x[..., :ROT].reshape(Bc, Sc, NH, ROT // 2, 2)
        x0, x1 = xr[..., 0], xr[..., 1]
        o0 = c_ * x0 - s_ * x1
        o1 = s_ * x0 + c_ * x1
        out = np.stack([o0, o1], axis=-1).reshape(Bc, Sc, NH, ROT)
        return np.concatenate([out, x[..., ROT:]], axis=-1)

    q, k = rot(q), rot(k)
    nk = np.asarray(layer_past_k, np.float32).copy()
    nv = np.asarray(layer_past_v, np.float32).copy()
    nk[np.asarray(new_key_loc)] = k.reshape(Bc * Sc, 1, NH, HD)
    nv[np.asarray(new_value_loc)] = v.reshape(Bc * Sc, 1, NH, HD)
    kg = nk[np.asarray(valid_key_indices)].reshape(
        Bc, bucket_size, NH, HD).transpose(0, 2, 1, 3)
    vg = nv[np.asarray(valid_value_indices)].reshape(
        Bc, bucket_size, NH, HD).transpose(0, 2, 1, 3)
    qh = q.transpose(0, 2, 1, 3)
    scores = np.einsum("bhqd,bhkd->bhqk", qh, kg)
    causal = np.tril(np.ones((MAX_POS, MAX_POS), bool))[
        bucket_size - Sc:bucket_size, :bucket_size]
    scores = np.where(causal, scores, np.float32(np.finfo(np.float32).min))
    scores = scores / np.float32(np.sqrt(HD)) + np.asarray(attention_mask, np.float32)
    scores = scores - scores.max(-1, keepdims=True)
    p = np.exp(scores)
    p = p / p.sum(-1, keepdims=True)
    attn = np.einsum("bhqk,bhkd->bhqd", p, vg)
    attn = attn.transpose(0, 2, 1, 3).reshape(Bc, Sc, Dc)
    return (attn @ np.asarray(Wo).T).astype(np.float32)


def _fast_path_ok(layer_past_k, layer_past_v, attention_mask, new_key_loc,
                  new_value_loc, valid_key_indices, valid_value_indices,
                  bucket_size, hs_shape):
    Bc, Sc, Dc = hs_shape
    if (Bc, Sc, Dc) != (B, S, D) or int(bucket_size) != S:
        return False
    ar = np.arange(Bc * Sc)
    for idx in (new_key_loc, new_value_loc, valid_key_indices, valid_value_indices):
        a = np.asarray(idx)
        if a.shape != (Bc * Sc,) or not np.array_equal(a, ar):
            return False
    if np.any(np.asarray(attention_mask) != 0):
        return False
    return True


_NC_CACHE = {}


def _get_nc(use_collective=True):
    key = ("v2", use_collective)
    if key not in _NC_CACHE:
        _NC_CACHE[key] = build_nc(_cfg_full(), use_collective=use_collective,
                                  n_cores=N_CORES)
    return _NC_CACHE[key]


def _assemble(outs, use_collective):
    # core r returns y^T [D, B*TPC]; its tokens are b*S + r*TPC + i
    TPC = S // len(outs)
    stk = np.stack([np.asarray(o, np.float32) for o in outs])  # [R, D, B*TPC]
    R = stk.shape[0]
    y = stk.reshape(R, D, B, TPC).transpose(2, 0, 3, 1).reshape(B * S, D)
    return y


def kernel(**inputs):
    hs = np.asarray(inputs["hidden_states"], np.float32)
    fast = _fast_path_ok(
        inputs["layer_past_k"], inputs["layer_past_v"], inputs["attention_mask"],
        inputs["new_key_loc"], inputs["new_value_loc"],
        inputs["valid_key_indices"], inputs["valid_value_indices"],
        inputs["bucket_size"], hs.shape)
    if not fast:
        return _numpy_reference(**inputs)

    from concourse.bass_utils import run_bass_kernel_spmd

    use_collective = os.environ.get("KERNEL_NO_COLLECTIVE", "") != "1"
    nc = _get_nc(use_collective)
    in_maps = _host_arrays(
        hs, np.asarray(inputs["Wq"], np.float32),
        np.asarray(inputs["Wk"], np.float32),
        np.asarray(inputs["Wv"], np.float32),
        np.asarray(inputs["Wo"], np.float32),
        inputs["position_ids"], _cfg_full(), N_CORES)
    res = run_bass_kernel_spmd(nc, in_maps, list(range(N_CORES)))
    outs = [res.results[c]["y"] for c in range(N_CORES)]
    y = _assemble(outs, use_collective)
    return y.reshape(B, S, D).astype(np.float32)


# revision 27
# speedup vs baseline: 1.0060x; 1.0060x over previous
"""GPT-J attention (B=2, S=2048, D=4096, 16 heads x 256, partial RoPE 64) on 8 trn2 cores.

Tensor-parallel attention + AllToAll-resharded out-projection (all matmul
inputs bf16, PSUM accumulation fp32):
  - Phase A per b: each core computes Q/K/V for its 2 heads; projections
    accumulate over all 32 d-chunks directly in PSUM. Five passes per
    512-token tile (Q01, Q23, K01, K23, V) using <= 6 PSUM banks with
    evacuation (alternating ACT/DVE) pipelined behind the next pass.
    Weights and hsT streamed in host-pre-swizzled partition-contiguous
    blocks. RoPE folded in per tile via a pair-swap PE matmul.
  - Phase B: scores computed TRANSPOSED (ssT[k,q] = KT_chunk^T @ QT) so the
    exp output is directly P^T, feeding PV with V as stationary - no PE
    transposes, no P copies. Row sums via a ones-column matmul; softmax
    normalization applied to the attention output in-place (PE outer-product
    broadcast of the reciprocal row sums + DVE multiply). Score pipeline is
    2 deep (3 PSUM score banks) so exp latency never stalls the PE.
  - AllToAll per b: each core ships its heads' normalized attention columns
    for the 256 tokens owned by each peer (4MB total wire vs 28MB for a
    y-partials ReduceScatter). b=0's exchange overlaps A(b=1)/B(b=1); the
    receive DMAs are emitted only at points where the collective is already
    done, so no in-order DMA queue ever blocks on it.
  - Phase C': each core out-projects only its own 512 tokens over the FULL
    4096 features, transposed (y^T[d, tok] = Wo_chunk^T @ x) so the full Wo
    streams through SBUF exactly once. Two half-token passes: the first
    starts right after B(b=1) and hides the b=1 AllToAll. Output is y^T
    per core; the host reassembles token shards.
"""

import os
import sys

import numpy as np

sys.path.insert(0, "/opt/trn_rl_repo")

# ---------------------------------------------------------------- constants
B = 2
S = 2048
D = 4096
NH = 16
HD = 256
ROT = 64
MAX_POS = 2048
N_CORES = 8
HPC = NH // N_CORES          # heads per core = 2
HDL = HPC * HD               # local head width = 512

SC = 512                     # token tile / q macro tile / moving width
NEG = -1.0e30


def _cfg_full():
    return dict(B=B, S=S, D=D, HPC=HPC, HD=HD, ROT=ROT)


# ---------------------------------------------------------------- bass build

def build_nc(cfg, use_collective=True, n_cores=N_CORES):
    import concourse.tile as tile
    from concourse import bacc, mybir

    fp32 = mybir.dt.float32
    bf16 = mybir.dt.bfloat16

    Bc, Sc, Dc, HPCc, HDc, ROTc = (
        cfg["B"], cfg["S"], cfg["D"], cfg["HPC"], cfg["HD"], cfg["ROT"])
    HDLc = HPCc * HDc                    # local head width (512)
    NHC = HDLc // 128                    # local hd chunks (4)
    NSC = Sc // SC                       # 512-token tiles per b (4)
    NDC = Dc // 128                      # d chunks (32)
    NG = NDC // 8                        # streamed weight groups (4)
    NOC = Dc // SC                       # out-proj column chunks (8)
    NKC = Sc // 128                      # k chunks per b (16)
    SHARD = (Bc * Sc) // n_cores if use_collective else Bc * Sc

    nc = bacc.Bacc(num_devices=n_cores)

    # inputs (per-core, host-pre-swizzled for contiguous DMA)
    hs_e = nc.declare_dram_parameter("hs_s", [Bc, NSC, 4, 128, NDC // 4, SC],
                                     bf16, isOutput=False)
    wq_e = nc.declare_dram_parameter("wq_s", [2, NG, 128, 8, HDLc // 2],
                                     bf16, isOutput=False)
    wk_e = nc.declare_dram_parameter("wk_s", [2, NG, 128, 8, HDLc // 2],
                                     bf16, isOutput=False)
    wv_e = nc.declare_dram_parameter("wv_s", [NG, 128, 8, HDLc],
                                     bf16, isOutput=False)
    wo_e = nc.declare_dram_parameter("wo_s", [NDC, 128, NDC, 128],
                                     bf16, isOutput=False)
    cos_e = nc.declare_dram_parameter("cosb", [Bc, ROTc, Sc], bf16, isOutput=False)
    sin_e = nc.declare_dram_parameter("sinb", [Bc, ROTc, Sc], bf16, isOutput=False)
    msk_e = nc.declare_dram_parameter("masksT", [128, 4, SC], bf16, isOutput=False)
    psw_e = nc.declare_dram_parameter("pswap", [128, ROTc], bf16, isOutput=False)
    one_e = nc.declare_dram_parameter("onesc", [128, 1], bf16, isOutput=False)
    onr_e = nc.declare_dram_parameter("onesr", [1, 128], fp32, isOutput=False)

    TPC = Sc // n_cores                  # tokens per core per batch (256)
    y_e = nc.declare_dram_parameter("y", [Dc, Bc * TPC], bf16, isOutput=True)
    a2a_in = [nc.dram_tensor(f"a2a_in{b}", [n_cores, 128, NHC, TPC], bf16)
              for b in range(Bc)]
    a2a_out = [nc.dram_tensor(f"a2a_out{b}", [n_cores, 128, NHC, TPC], bf16)
               for b in range(Bc)]

    def mm(ps, lhsT, rhs, start, stop):
        nc.tensor.matmul(ps, lhsT, rhs, start=start, stop=stop)

    with tile.TileContext(nc) as tc:
        with tc.tile_pool(name="const", bufs=1) as constp:
            masks = constp.tile([128, 4, SC], bf16)
            nc.sync.dma_start(masks[:], msk_e[:])
            pswap = constp.tile([128, ROTc], bf16)
            nc.sync.dma_start(pswap[:], psw_e[:])
            ones = constp.tile([128, 1], bf16)
            nc.sync.dma_start(ones[:], one_e[:])
            onesr = constp.tile([1, 128], fp32)
            nc.sync.dma_start(onesr[:], onr_e[:])

            with (
                tc.tile_pool(name="qkv", bufs=1) as qkvp,      # QT/KT/V one b
                tc.tile_pool(name="atn", bufs=1) as atnp,      # ATN one b
                tc.tile_pool(name="xn", bufs=1) as xnp,        # received x
                tc.tile_pool(name="bcs", bufs=2) as bcsb,
                tc.tile_pool(name="bcps", bufs=1, space="PSUM") as bcps,
            ):
                xn = [xnp.tile([128, NHC, Bc * TPC], bf16, tag=f"xn{s}",
                               name=f"xn{s}") for s in range(n_cores)]

                def emit_recv(b):
                    # receive pre-normalized xn columns for batch half b
                    bsl = slice(b * TPC, (b + 1) * TPC)
                    for s in range(n_cores):
                        nc.sync.dma_start(xn[s][:, :, bsl], a2a_out[b][s])

                for b in range(Bc):
                    # ============ phase A: QKV projection (PSUM-resident) ====
                    QT = [qkvp.tile([128, Sc], bf16, tag=f"QT{c}", name=f"QT{c}") for c in range(NHC)]
                    KT = [qkvp.tile([128, Sc], bf16, tag=f"KT{c}", name=f"KT{c}") for c in range(NHC)]
                    V = [qkvp.tile([128, HDLc], bf16, tag=f"V{k}", name=f"V{k}") for k in range(NKC)]

                    with (
                        tc.tile_pool(name="trig", bufs=1) as trigp,
                        tc.tile_pool(name="hst", bufs=6) as hp,
                        tc.tile_pool(name="wqk", bufs=5) as wqkp,
                        tc.tile_pool(name="wvs", bufs=3) as wvp,
                        tc.tile_pool(name="pjps", bufs=1, space="PSUM") as pjps,
                        tc.tile_pool(name="rops", bufs=2, space="PSUM") as ropsp,
                        tc.tile_pool(name="ropb", bufs=1) as ropbp,
                    ):
                        cosb = trigp.tile([ROTc, Sc], bf16, tag="cos")
                        sinb = trigp.tile([ROTc, Sc], bf16, tag="sin")
                        nc.sync.dma_start(cosb[:], cos_e[b])
                        nc.sync.dma_start(sinb[:], sin_e[b])
                        HQD = NDC // 4
                        for st in range(NSC):
                            ssl = slice(st * SC, (st + 1) * SC)
                            hq = []
                            for q4 in range(4):
                                hq.append(hp.tile([128, HQD, SC], bf16,
                                                  tag="hst", name="hst"))
                                nc.sync.dma_start(hq[q4][:], hs_e[b, st, q4])

                            def hst(dc, _hq=hq):
                                return _hq[dc // HQD][:, dc % HQD, :]

                            def rope(t, c):
                                # rotate rows 0:ROT of t[c] at columns ssl
                                sw = ropsp.tile([ROTc, SC], fp32, tag="rp")
                                mm(sw[:], pswap[:, :], t[c][:, ssl],
                                   start=True, stop=True)
                                t1 = ropbp.tile([ROTc, SC], bf16, tag="t1")
                                t2 = ropbp.tile([ROTc, SC], bf16, tag="t2")
                                nc.vector.tensor_tensor(
                                    t1[:], sw[:], sinb[:, ssl],
                                    op=mybir.AluOpType.mult)
                                nc.vector.tensor_tensor(
                                    t2[:], t[c][0:ROTc, ssl], cosb[:, ssl],
                                    op=mybir.AluOpType.mult)
                                nc.vector.tensor_add(
                                    t[c][0:ROTc, ssl], t1[:], t2[:])

                            # 4 Q/K passes (2 banks each) + 1 V pass (4 banks)
                            for pi, (we, dst, hf) in enumerate((
                                    (wq_e, QT, 0), (wq_e, QT, 1),
                                    (wk_e, KT, 0), (wk_e, KT, 1))):
                                bk = (pi % 2) * 2
                                t0 = pjps.tile([128, SC], fp32, tag=f"pj{bk}")
                                t1_ = pjps.tile([128, SC], fp32, tag=f"pj{bk + 1}")
                                for g in range(NG):
                                    wa = wqkp.tile([128, 8, HDLc // 2], bf16,
                                                   tag="wa")
                                    nc.sync.dma_start(wa[:], we[hf, g])
                                    for j in range(8):
                                        dc = g * 8 + j
                                        st_, sp_ = (dc == 0), (dc == NDC - 1)
                                        mm(t0[:], wa[:, j, 0:128], hst(dc),
                                           start=st_, stop=sp_)
                                        mm(t1_[:], wa[:, j, 128:256], hst(dc),
                                           start=st_, stop=sp_)
                                for j, ps in enumerate((t0, t1_)):
                                    c = hf * 2 + j
                                    if pi % 2 == 0:
                                        nc.scalar.copy(dst[c][:, ssl], ps[:])
                                    else:
                                        nc.vector.tensor_copy(dst[c][:, ssl], ps[:])
                                if hf == 0:
                                    rope(dst, 0)
                                else:
                                    rope(dst, 2)

                            # V pass: stationary = hst chunks, moving = wv
                            psv = [pjps.tile([128, HDLc], fp32, tag=f"pj{ss}", name=f"psv{ss}")
                                   for ss in range(4)]
                            for g in range(NG):
                                wvt = wvp.tile([128, 8, HDLc], bf16, tag="wv")
                                nc.sync.dma_start(wvt[:], wv_e[g])
                                for j in range(8):
                                    dc = g * 8 + j
                                    st_, sp_ = (dc == 0), (dc == NDC - 1)
                                    for ss in range(4):
                                        mm(psv[ss][:],
                                           hst(dc)[:, ss * 128:(ss + 1) * 128],
                                           wvt[:, j, :], start=st_, stop=sp_)
                            for ss in range(4):
                                kcv = st * 4 + ss
                                if ss % 2 == 0:
                                    nc.scalar.copy(V[kcv][:], psv[ss][:])
                                else:
                                    nc.vector.tensor_copy(V[kcv][:], psv[ss][:])

                    # ============ phase B: attention (transposed scores) =====
                    if b == 1:
                        emit_recv(0)
                    ATN = atnp.tile([128, NHC, Sc], bf16, tag="ATN", name="ATN")
                    
                    with (
                        tc.tile_pool(name="ptb", bufs=1) as ptp,
                        tc.tile_pool(name="rsb", bufs=1) as rsbp,
                        tc.tile_pool(name="scps", bufs=1, space="PSUM") as scps,
                        tc.tile_pool(name="atps", bufs=1, space="PSUM") as atps,
                        tc.tile_pool(name="rsps", bufs=1, space="PSUM") as rsps,
                    ):
                        psRS = rsps.tile([128, SC], fp32, tag="rs0")
                        rrb = rsbp.tile([1, 8, SC], fp32, tag="rrec")

                        def emit_scores(h, qm, kc):
                            c0 = h * (HDc // 128)
                            qsl = slice(qm * SC, (qm + 1) * SC)
                            kcl = slice(kc * 128, (kc + 1) * 128)
                            ss = scps.tile([128, SC], fp32, tag=f"ss{kc % 3}",
                                           name=f"ss{kc % 3}")
                            mm(ss[:], KT[c0][:, kcl], QT[c0][:, qsl],
                               start=True, stop=False)
                            mm(ss[:], KT[c0 + 1][:, kcl], QT[c0 + 1][:, qsl],
                               start=False, stop=True)
                            return ss

                        for h in range(HPCc):
                            c0 = h * (HDc // 128)
                            for qm in range(NSC):
                                nkc = (qm + 1) * 4
                                at = [atps.tile([128, SC], fp32, tag=f"at{hh}", name=f"at{hh}")
                                      for hh in range(HDc // 128)]
                                ss_cur = emit_scores(h, qm, 0)
                                ss_nxt = (emit_scores(h, qm, 1)
                                          if nkc > 1 else None)
                                for kc in range(nkc):
                                    if kc // 4 == qm:   # diagonal macro tile
                                        nc.vector.tensor_add(
                                            ss_cur[:], ss_cur[:],
                                            masks[:, kc % 4, :])
                                    pt = ptp.tile([128, SC], bf16,
                                                  tag=f"pt{kc % 3}")
                                    nc.scalar.activation(
                                        pt[:], ss_cur[:],
                                        mybir.ActivationFunctionType.Exp,
                                        bias=0.0, scale=1.0 / 16.0)
                                    ss_fut = (emit_scores(h, qm, kc + 2)
                                              if kc + 2 < nkc else None)
                                    ss_cur, ss_nxt = ss_nxt, ss_fut
                                    st_, sp_ = (kc == 0), (kc == nkc - 1)
                                    for hh in range(HDc // 128):
                                        mm(at[hh][:],
                                           V[kc][:, h * HDc + hh * 128:
                                                 h * HDc + (hh + 1) * 128],
                                           pt[:], start=st_, stop=sp_)
                                    mm(psRS[0:1, :],
                                       ones[:], pt[:], start=st_, stop=sp_)
                                nc.vector.reciprocal(
                                    rrb[0:1, h * NSC + qm, :], psRS[0:1, :])
                                qsl = slice(qm * SC, (qm + 1) * SC)
                                for hh in range(HDc // 128):
                                    if hh == 0:
                                        nc.scalar.copy(ATN[:, c0 + hh, qsl],
                                                       at[hh][:])
                                    else:
                                        nc.vector.tensor_copy(
                                            ATN[:, c0 + hh, qsl], at[hh][:])
                                i2 = (h * NSC + qm) % 2
                                bc_ps = bcps.tile([128, SC], fp32,
                                                  tag=f"bc{i2}", name=f"bc{i2}")
                                mm(bc_ps[:], onesr[:],
                                   rrb[0:1, h * NSC + qm, :],
                                   start=True, stop=True)
                                bcs = bcsb.tile([128, SC], bf16, tag=f"bcs{i2}",
                                                name=f"bcs{i2}")
                                nc.vector.tensor_copy(bcs[:], bc_ps[:])
                                for hh in range(HDc // 128):
                                    nc.vector.tensor_tensor(
                                        ATN[:, c0 + hh, qsl],
                                        ATN[:, c0 + hh, qsl],
                                        bcs[:], op=mybir.AluOpType.mult)

                        # ship ATN token-slices + recips to owning cores
                        for r in range(n_cores):
                            nc.scalar.dma_start(
                                a2a_in[b][r],
                                ATN[:, :, r * TPC:(r + 1) * TPC])
                        if use_collective:
                            nc.gpsimd.collective_compute(
                                "AllToAll", mybir.AluOpType.bypass,
                                replica_groups=[list(range(n_cores))],
                                ins=[a2a_in[b][:]], outs=[a2a_out[b][:]])
                        else:
                            nc.sync.dma_start(a2a_out[b][:], a2a_in[b][:])


                # ===== phase C': local out-projection over owned tokens ==
                # two half-token passes: half 0 (b=0 columns) starts right
                # after B(b1) and hides the b=1 AllToAll; half 1 follows
                with (
                    tc.tile_pool(name="wo2", bufs=6) as wo2p,
                    tc.tile_pool(name="ysbT", bufs=2) as ysbtp,
                    tc.tile_pool(name="ypsT", bufs=2, space="PSUM") as ypstp,
                ):
                    for half in range(Bc):
                        if half == 1:
                            emit_recv(1)
                        hsl = slice(half * TPC, (half + 1) * TPC)
                        for dsub in range(NDC):
                            wo2 = wo2p.tile([128, NDC, 128], bf16, tag="wo2")
                            nc.sync.dma_start(wo2[:], wo_e[dsub])
                            yT = ypstp.tile([128, TPC], fp32,
                                            tag=f"yT{dsub % 2}",
                                            name=f"yT{dsub % 2}")
                            for cg in range(NDC):
                                s, c = cg // NHC, cg % NHC
                                mm(yT[:], wo2[:, cg, :], xn[s][:, c, hsl],
                                   start=(cg == 0), stop=(cg == NDC - 1))
                            ysbT = ysbtp.tile([128, TPC], bf16, tag="ysbT")
                            if dsub % 2 == 0:
                                nc.scalar.copy(ysbT[:], yT[:])
                            else:
                                nc.vector.tensor_copy(ysbT[:], yT[:])
                            nc.gpsimd.dma_start(
                                y_e[dsub * 128:(dsub + 1) * 128, hsl],
                                ysbT[:])

    nc.compile()
    return nc


# ---------------------------------------------------------------- host prep

def _sinusoidal_np(num_pos, dim):
    inv_freq = 1.0 / (10000.0 ** (np.arange(0, dim, 2, dtype=np.float32) / dim))
    t = np.arange(num_pos, dtype=np.float32)[:, None] * inv_freq[None, :]
    return np.cos(t).astype(np.float32), np.sin(t).astype(np.float32)


def _host_arrays(hs, Wq, Wk, Wv, Wo, position_ids, cfg, n_cores):
    """Build the shared + per-core input arrays (pre-swizzled, bf16)."""
    import ml_dtypes
    bf = ml_dtypes.bfloat16
    Bc, Sc, Dc, HPCc, HDc, ROTc = (
        cfg["B"], cfg["S"], cfg["D"], cfg["HPC"], cfg["HD"], cfg["ROT"])
    HDLc = HPCc * HDc
    NSCc, NDCc, NGc, NOCc, NHCc = Sc // SC, Dc // 128, Dc // 1024, Dc // SC, HDLc // 128

    # hs_s[b, st, hf, p, j, f] = hs[b, st*SC+f, (hf*16+j)*128+p]
    hs_s = np.ascontiguousarray(
        hs.reshape(Bc, NSCc, SC, 4, NDCc // 4, 128)
        .transpose(0, 1, 3, 5, 4, 2)).astype(bf)

    cos_t, sin_t = _sinusoidal_np(max(MAX_POS, Sc), ROTc)   # [P, ROT//2]
    pos = np.asarray(position_ids).astype(np.int64)         # [B, S]
    cosg = cos_t[pos]                                       # [B, S, 32]
    sing = sin_t[pos]
    cosb = np.repeat(cosg.transpose(0, 2, 1), 2, axis=1)    # [B, 64, S]
    sinb_r = np.repeat(sing.transpose(0, 2, 1), 2, axis=1)
    sgn = np.ones((ROTc, 1), np.float32)
    sgn[0::2] = -1.0
    sinb = np.ascontiguousarray(sinb_r * sgn).astype(bf)
    cosb = np.ascontiguousarray(cosb).astype(bf)

    # transposed causal masks for diagonal 512 macro tile: masksT[k, kc, q]
    masksT = np.zeros((128, 4, SC), np.float32)
    kk = np.arange(128)[:, None]
    qq = np.arange(SC)[None, :]
    for m in range(4):
        masksT[:, m, :] = np.where(m * 128 + kk <= qq, 0.0, NEG)
    masksT = masksT.astype(bf)

    pswap = np.zeros((128, ROTc), np.float32)
    for f in range(ROTc // 2):
        pswap[2 * f + 1, 2 * f] = 1.0
        pswap[2 * f, 2 * f + 1] = 1.0
    onesc = np.ones((128, 1), np.float32).astype(bf)
    onesr = np.ones((1, 128), np.float32)

    # wo2_s[dsub, p, cg, m] = Wo[dsub*128+m, cg*128+p] (full Wo, shared)
    wo2_s = np.ascontiguousarray(
        np.asarray(Wo).reshape(NDCc, 128, NDCc, 128)
        .transpose(0, 3, 2, 1)).astype(bf)

    shared = dict(hs_s=hs_s, cosb=cosb, sinb=sinb, masksT=masksT,
                  pswap=pswap.astype(bf), onesc=onesc, onesr=onesr,
                  wo_s=wo2_s)

    def _wswz_qk(w):   # [HDLc(rows of W slice), Dc] -> [2, NG, 128, 8, HDLc//2]
        # w here is the [HDLc, Dc] row-slice of the full weight; stationary
        # layout wq_s[hf, g, p, j, f] = w[hf*256+f, (g*8+j)*128+p]
        return np.ascontiguousarray(
            w.reshape(2, HDLc // 2, NGc, 8, 128)
            .transpose(0, 2, 4, 3, 1)).astype(bf)

    def _wswz_v(w):    # -> [NG, 128, 8, HDLc];  wv_s[g,p,j,f] = w[f,(g*8+j)*128+p]
        return np.ascontiguousarray(
            w.reshape(HDLc, NGc, 8, 128).transpose(1, 3, 2, 0)).astype(bf)

    per_core = []
    for c in range(n_cores):
        csl = slice(c * HDLc, (c + 1) * HDLc)
        per_core.append(dict(
            wq_s=_wswz_qk(np.asarray(Wq)[csl, :]),
            wk_s=_wswz_qk(np.asarray(Wk)[csl, :]),
            wv_s=_wswz_v(np.asarray(Wv)[csl, :]),
            **shared,
        ))
    return per_core


def _numpy_reference(hidden_states, Wq, Wk, Wv, Wo, layer_past_k, layer_past_v,
                     attention_mask, position_ids, new_key_loc, new_value_loc,
                     valid_key_indices, valid_value_indices, bucket_size):
    """Slow but general fallback (mirrors reference.py in numpy fp32)."""
    hs = np.asarray(hidden_states, np.float32)
    Bc, Sc, Dc = hs.shape
    q = (hs @ np.asarray(Wq).T).reshape(Bc, Sc, NH, HD)
    k = (hs @ np.asarray(Wk).T).reshape(Bc, Sc, NH, HD)
    v = (hs @ np.asarray(Wv).T).reshape(Bc, Sc, NH, HD)

    cos_t, sin_t = _sinusoidal_np(MAX_POS, ROT)
    pos = np.asarray(position_ids).astype(np.int64)
    c_ = cos_t[pos][:, :, None, :]      # [B,S,1,32]
    s_ = sin_t[pos][:, :, None, :]

    def rot(x):
        xr = x[..., :ROT].reshape(Bc, Sc, NH, ROT // 2, 2)
        x0, x1 = xr[..., 0], xr[..., 1]
        o0 = c_ * x0 - s_ * x1
        o1 = s_ * x0 + c_ * x1
        out = np.stack([o0, o1], axis=-1).reshape(Bc, Sc, NH, ROT)
        return np.concatenate([out, x[..., ROT:]], axis=-1)

    q, k = rot(q), rot(k)
    nk = np.asarray(layer_past_k, np.float32).copy()
    nv = np.asarray(layer_past_v, np.float32).copy()
    nk[np.asarray(new_key_loc)] = k.reshape(Bc * Sc, 1, NH, HD)
    nv[np.asarray(new_value_loc)] = v.reshape(Bc * Sc, 1, NH, HD)
    kg = nk[np.asarray(valid_key_indices)].reshape(
        Bc, bucket_size, NH, HD).transpose(0, 2, 1, 3)
    vg = nv[np.asarray(valid_value_indices)].reshape(
        Bc, bucket_size, NH, HD).transpose(0, 2, 1, 3)
    qh = q.transpose(0, 2, 1, 3)
    scores = np.einsum("bhqd,bhkd->bhqk", qh, kg)
    causal = np.tril(np.ones((MAX_POS, MAX_POS), bool))[
        bucket_size - Sc:bucket_size, :bucket_size]
    scores = np.where(causal, scores, np.float32(np.finfo(np.float32).min))
    scores = scores / np.float32(np.sqrt(HD)) + np.asarray(attention_mask, np.float32)
    scores = scores - scores.max(-1, keepdims=True)
    p = np.exp(scores)
    p = p / p.sum(-1, keepdims=True)
    attn = np.einsum("bhqk,bhkd->bhqd", p, vg)
    attn = attn.transpose(0, 2, 1, 3).reshape(Bc, Sc, Dc)
    return (attn @ np.asarray(Wo).T).astype(np.float32)


def _fast_path_ok(layer_past_k, layer_past_v, attention_mask, new_key_loc,
                  new_value_loc, valid_key_indices, valid_value_indices,
                  bucket_size, hs_shape):
    Bc, Sc, Dc = hs_shape
    if (Bc, Sc, Dc) != (B, S, D) or int(bucket_size) != S:
        return False
    ar = np.arange(Bc * Sc)
    for idx in (new_key_loc, new_value_loc, valid_key_indices, valid_value_indices):
        a = np.asarray(idx)
        if a.shape != (Bc * Sc,) or not np.array_equal(a, ar):
            return False
    if np.any(np.asarray(attention_mask) != 0):
        return False
    return True


_NC_CACHE = {}


def _get_nc(use_collective=True):
    key = ("v2", use_collective)
    if key not in _NC_CACHE:
        _NC_CACHE[key] = build_nc(_cfg_full(), use_collective=use_collective,
                                  n_cores=N_CORES)
    return _NC_CACHE[key]


def _assemble(outs, use_collective):
    # core r returns y^T [D, B*TPC]; its tokens are b*S + r*TPC + i
    TPC = S // len(outs)
    stk = np.stack([np.asarray(o, np.float32) for o in outs])  # [R, D, B*TPC]
    R = stk.shape[0]
    y = stk.reshape(R, D, B, TPC).transpose(2, 0, 3, 1).reshape(B * S, D)
    return y


def kernel(**inputs):
    hs = np.asarray(inputs["hidden_states"], np.float32)
    fast = _fast_path_ok(
        inputs["layer_past_k"], inputs["layer_past_v"], inputs["attention_mask"],
        inputs["new_key_loc"], inputs["new_value_loc"],
        inputs["valid_key_indices"], inputs["valid_value_indices"],
        inputs["bucket_size"], hs.shape)
    if not fast:
        return _numpy_reference(**inputs)

    from concourse.bass_utils import run_bass_kernel_spmd

    use_collective = os.environ.get("KERNEL_NO_COLLECTIVE", "") != "1"
    nc = _get_nc(use_collective)
    in_maps = _host_arrays(
        hs, np.asarray(inputs["Wq"], np.float32),
        np.asarray(inputs["Wk"], np.float32),
        np.asarray(inputs["Wv"], np.float32),
        np.asarray(inputs["Wo"], np.float32),
        inputs["position_ids"], _cfg_full(), N_CORES)
    res = run_bass_kernel_spmd(nc, in_maps, list(range(N_CORES)))
    outs = [res.results[c]["y"] for c in range(N_CORES)]
    y = _assemble(outs, use_collective)
    return y.reshape(B, S, D).astype(np.float32)


# revision 30
# speedup vs baseline: 1.0219x; 1.0157x over previous
"""GPT-J attention (B=2, S=2048, D=4096, 16 heads x 256, partial RoPE 64) on 8 trn2 cores.

Tensor-parallel attention + AllToAll-resharded out-projection (all matmul
inputs bf16, PSUM accumulation fp32):
  - Phase A per b: each core computes Q/K/V for its 2 heads; projections
    accumulate over all 32 d-chunks directly in PSUM. Five passes per
    512-token tile (Q01, Q23, K01, K23, V) using <= 6 PSUM banks with
    evacuation (alternating ACT/DVE) pipelined behind the next pass.
    Weights and hsT streamed in host-pre-swizzled partition-contiguous
    blocks. RoPE folded in per tile via a pair-swap PE matmul.
  - Phase B: scores computed TRANSPOSED (ssT[k,q] = KT_chunk^T @ QT) so the
    exp output is directly P^T, feeding PV with V as stationary - no PE
    transposes, no P copies. Row sums via a ones-column matmul; softmax
    normalization applied to the attention output in-place (PE outer-product
    broadcast of the reciprocal row sums + DVE multiply). Score pipeline is
    2 deep (3 PSUM score banks) so exp latency never stalls the PE.
  - AllToAll per b: each core ships its heads' normalized attention columns
    for the 256 tokens owned by each peer (4MB total wire vs 28MB for a
    y-partials ReduceScatter). b=0's exchange overlaps A(b=1)/B(b=1); the
    receive DMAs are emitted only at points where the collective is already
    done, so no in-order DMA queue ever blocks on it.
  - Phase C': each core out-projects only its own 512 tokens over the FULL
    4096 features, transposed (y^T[d, tok] = Wo_chunk^T @ x) so the full Wo
    streams through SBUF exactly once. Two half-token passes: the first
    starts right after B(b=1) and hides the b=1 AllToAll. Output is y^T
    per core; the host reassembles token shards.
"""

import os
import sys

import numpy as np

sys.path.insert(0, "/opt/trn_rl_repo")

# ---------------------------------------------------------------- constants
B = 2
S = 2048
D = 4096
NH = 16
HD = 256
ROT = 64
MAX_POS = 2048
N_CORES = 8
HPC = NH // N_CORES          # heads per core = 2
HDL = HPC * HD               # local head width = 512

SC = 512                     # token tile / q macro tile / moving width
NEG = -1.0e30


def _cfg_full():
    return dict(B=B, S=S, D=D, HPC=HPC, HD=HD, ROT=ROT)


# ---------------------------------------------------------------- bass build

def build_nc(cfg, use_collective=True, n_cores=N_CORES):
    import concourse.tile as tile
    from concourse import bacc, mybir

    fp32 = mybir.dt.float32
    bf16 = mybir.dt.bfloat16

    Bc, Sc, Dc, HPCc, HDc, ROTc = (
        cfg["B"], cfg["S"], cfg["D"], cfg["HPC"], cfg["HD"], cfg["ROT"])
    HDLc = HPCc * HDc                    # local head width (512)
    NHC = HDLc // 128                    # local hd chunks (4)
    NSC = Sc // SC                       # 512-token tiles per b (4)
    NDC = Dc // 128                      # d chunks (32)
    NG = NDC // 8                        # streamed weight groups (4)
    NOC = Dc // SC                       # out-proj column chunks (8)
    NKC = Sc // 128                      # k chunks per b (16)
    SHARD = (Bc * Sc) // n_cores if use_collective else Bc * Sc

    nc = bacc.Bacc(num_devices=n_cores)

    # inputs (per-core, host-pre-swizzled for contiguous DMA)
    hs_e = nc.declare_dram_parameter("hs_s", [Bc, NSC, 4, 128, NDC // 4, SC],
                                     bf16, isOutput=False)
    wq_e = nc.declare_dram_parameter("wq_s", [2, NG, 128, 8, HDLc // 2],
                                     bf16, isOutput=False)
    wk_e = nc.declare_dram_parameter("wk_s", [2, NG, 128, 8, HDLc // 2],
                                     bf16, isOutput=False)
    wv_e = nc.declare_dram_parameter("wv_s", [NG, 128, 8, HDLc],
                                     bf16, isOutput=False)
    wo_e = nc.declare_dram_parameter("wo_s", [NDC, 128, NDC, 128],
                                     bf16, isOutput=False)
    cos_e = nc.declare_dram_parameter("cosb", [Bc, ROTc, Sc], bf16, isOutput=False)
    sin_e = nc.declare_dram_parameter("sinb", [Bc, ROTc, Sc], bf16, isOutput=False)
    msk_e = nc.declare_dram_parameter("masksT", [128, 4, SC], bf16, isOutput=False)
    psw_e = nc.declare_dram_parameter("pswap", [128, ROTc], bf16, isOutput=False)
    one_e = nc.declare_dram_parameter("onesc", [128, 1], bf16, isOutput=False)
    onr_e = nc.declare_dram_parameter("onesr", [1, 128], fp32, isOutput=False)

    TPC = Sc // n_cores                  # tokens per core per batch (256)
    y_e = nc.declare_dram_parameter("y", [Dc, Bc * TPC], bf16, isOutput=True)
    a2a_in = [nc.dram_tensor(f"a2a_in{b}", [n_cores, 128, NHC, TPC], bf16)
              for b in range(Bc)]
    a2a_out = [nc.dram_tensor(f"a2a_out{b}", [n_cores, 128, NHC, TPC], bf16)
               for b in range(Bc)]

    def mm(ps, lhsT, rhs, start, stop):
        nc.tensor.matmul(ps, lhsT, rhs, start=start, stop=stop)

    with tile.TileContext(nc) as tc:
        with tc.tile_pool(name="const", bufs=1) as constp:
            masks = constp.tile([128, 4, SC], bf16)
            nc.sync.dma_start(masks[:], msk_e[:])
            pswap = constp.tile([128, ROTc], bf16)
            nc.sync.dma_start(pswap[:], psw_e[:])
            ones = constp.tile([128, 1], bf16)
            nc.sync.dma_start(ones[:], one_e[:])
            onesr = constp.tile([1, 128], fp32)
            nc.sync.dma_start(onesr[:], onr_e[:])

            with (
                tc.tile_pool(name="qkv", bufs=1) as qkvp,      # QT/KT/V one b
                tc.tile_pool(name="atn", bufs=1) as atnp,      # ATN one b
                tc.tile_pool(name="xn", bufs=1) as xnp,        # received x
                tc.tile_pool(name="bcs", bufs=2) as bcsb,
                tc.tile_pool(name="bcps", bufs=1, space="PSUM") as bcps,
            ):
                xn = [xnp.tile([128, NHC, Bc * TPC], bf16, tag=f"xn{s}",
                               name=f"xn{s}") for s in range(n_cores)]

                def emit_recv(b):
                    # receive pre-normalized xn columns for batch half b
                    bsl = slice(b * TPC, (b + 1) * TPC)
                    for s in range(n_cores):
                        nc.sync.dma_start(xn[s][:, :, bsl], a2a_out[b][s])

                for b in range(Bc):
                    # ============ phase A: QKV projection (PSUM-resident) ====
                    QT = [qkvp.tile([128, Sc], bf16, tag=f"QT{c}", name=f"QT{c}") for c in range(NHC)]
                    KT = [qkvp.tile([128, Sc], bf16, tag=f"KT{c}", name=f"KT{c}") for c in range(NHC)]
                    V = [qkvp.tile([128, HDLc], bf16, tag=f"V{k}", name=f"V{k}") for k in range(NKC)]

                    with (
                        tc.tile_pool(name="trig", bufs=1) as trigp,
                        tc.tile_pool(name="hst", bufs=6) as hp,
                        tc.tile_pool(name="wqk", bufs=5) as wqkp,
                        tc.tile_pool(name="wvs", bufs=3) as wvp,
                        tc.tile_pool(name="pjps", bufs=1, space="PSUM") as pjps,
                        tc.tile_pool(name="rops", bufs=2, space="PSUM") as ropsp,
                        tc.tile_pool(name="ropb", bufs=1) as ropbp,
                    ):
                        cosb = trigp.tile([ROTc, Sc], bf16, tag="cos")
                        sinb = trigp.tile([ROTc, Sc], bf16, tag="sin")
                        nc.sync.dma_start(cosb[:], cos_e[b])
                        nc.sync.dma_start(sinb[:], sin_e[b])
                        HQD = NDC // 4
                        for st in range(NSC):
                            ssl = slice(st * SC, (st + 1) * SC)
                            hq = []
                            for q4 in range(4):
                                hq.append(hp.tile([128, HQD, SC], bf16,
                                                  tag="hst", name="hst"))
                                nc.sync.dma_start(hq[q4][:], hs_e[b, st, q4])

                            def hst(dc, _hq=hq):
                                return _hq[dc // HQD][:, dc % HQD, :]

                            def rope(t, c):
                                # rotate rows 0:ROT of t[c] at columns ssl
                                sw = ropsp.tile([ROTc, SC], fp32, tag="rp")
                                mm(sw[:], pswap[:, :], t[c][:, ssl],
                                   start=True, stop=True)
                                t1 = ropbp.tile([ROTc, SC], bf16, tag="t1")
                                t2 = ropbp.tile([ROTc, SC], bf16, tag="t2")
                                nc.vector.tensor_tensor(
                                    t1[:], sw[:], sinb[:, ssl],
                                    op=mybir.AluOpType.mult)
                                nc.vector.tensor_tensor(
                                    t2[:], t[c][0:ROTc, ssl], cosb[:, ssl],
                                    op=mybir.AluOpType.mult)
                                nc.vector.tensor_add(
                                    t[c][0:ROTc, ssl], t1[:], t2[:])

                            # 4 Q/K passes (2 banks each) + 1 V pass (4 banks)
                            for pi, (we, dst, hf) in enumerate((
                                    (wq_e, QT, 0), (wq_e, QT, 1),
                                    (wk_e, KT, 0), (wk_e, KT, 1))):
                                bk = (pi % 2) * 2
                                t0 = pjps.tile([128, SC], fp32, tag=f"pj{bk}")
                                t1_ = pjps.tile([128, SC], fp32, tag=f"pj{bk + 1}")
                                for g in range(NG):
                                    wa = wqkp.tile([128, 8, HDLc // 2], bf16,
                                                   tag="wa")
                                    nc.sync.dma_start(wa[:], we[hf, g])
                                    for j in range(8):
                                        dc = g * 8 + j
                                        st_, sp_ = (dc == 0), (dc == NDC - 1)
                                        mm(t0[:], wa[:, j, 0:128], hst(dc),
                                           start=st_, stop=sp_)
                                        mm(t1_[:], wa[:, j, 128:256], hst(dc),
                                           start=st_, stop=sp_)
                                for j, ps in enumerate((t0, t1_)):
                                    c = hf * 2 + j
                                    if pi % 2 == 0:
                                        nc.scalar.copy(dst[c][:, ssl], ps[:])
                                    else:
                                        nc.vector.tensor_copy(dst[c][:, ssl], ps[:])
                                if hf == 0:
                                    rope(dst, 0)
                                else:
                                    rope(dst, 2)

                            # V pass: stationary = hst chunks, moving = wv
                            psv = [pjps.tile([128, HDLc], fp32, tag=f"pj{ss}", name=f"psv{ss}")
                                   for ss in range(4)]
                            for g in range(NG):
                                wvt = wvp.tile([128, 8, HDLc], bf16, tag="wv")
                                nc.sync.dma_start(wvt[:], wv_e[g])
                                for j in range(8):
                                    dc = g * 8 + j
                                    st_, sp_ = (dc == 0), (dc == NDC - 1)
                                    for ss in range(4):
                                        mm(psv[ss][:],
                                           hst(dc)[:, ss * 128:(ss + 1) * 128],
                                           wvt[:, j, :], start=st_, stop=sp_)
                            for ss in range(4):
                                kcv = st * 4 + ss
                                if ss % 2 == 0:
                                    nc.scalar.copy(V[kcv][:], psv[ss][:])
                                else:
                                    nc.vector.tensor_copy(V[kcv][:], psv[ss][:])

                    # ============ phase B: attention (transposed scores) =====
                    if b == 1:
                        emit_recv(0)
                    ATN = atnp.tile([128, NHC, Sc], bf16, tag="ATN", name="ATN")
                    
                    with (
                        tc.tile_pool(name="ptb", bufs=1) as ptp,
                        tc.tile_pool(name="rsb", bufs=1) as rsbp,
                        tc.tile_pool(name="scps", bufs=1, space="PSUM") as scps,
                        tc.tile_pool(name="atps", bufs=1, space="PSUM") as atps,
                        tc.tile_pool(name="rsps", bufs=1, space="PSUM") as rsps,
                    ):
                        psRS = rsps.tile([128, SC], fp32, tag="rs0")
                        rrb = rsbp.tile([1, 8, SC], fp32, tag="rrec")

                        def emit_scores(h, qm, kc):
                            c0 = h * (HDc // 128)
                            qsl = slice(qm * SC, (qm + 1) * SC)
                            kcl = slice(kc * 128, (kc + 1) * 128)
                            ss = scps.tile([128, SC], fp32, tag=f"ss{kc % 3}",
                                           name=f"ss{kc % 3}")
                            mm(ss[:], KT[c0][:, kcl], QT[c0][:, qsl],
                               start=True, stop=False)
                            mm(ss[:], KT[c0 + 1][:, kcl], QT[c0 + 1][:, qsl],
                               start=False, stop=True)
                            return ss

                        for h in range(HPCc):
                            c0 = h * (HDc // 128)
                            for qm in range(NSC):
                                nkc = (qm + 1) * 4
                                at = [atps.tile([128, SC], fp32, tag=f"at{hh}", name=f"at{hh}")
                                      for hh in range(HDc // 128)]
                                ss_cur = emit_scores(h, qm, 0)
                                ss_nxt = (emit_scores(h, qm, 1)
                                          if nkc > 1 else None)
                                for kc in range(nkc):
                                    if kc // 4 == qm:   # diagonal macro tile
                                        nc.vector.tensor_add(
                                            ss_cur[:], ss_cur[:],
                                            masks[:, kc % 4, :])
                                    pt = ptp.tile([128, SC], bf16,
                                                  tag=f"pt{kc % 3}")
                                    nc.scalar.activation(
                                        pt[:], ss_cur[:],
                                        mybir.ActivationFunctionType.Exp,
                                        bias=0.0, scale=1.0 / 16.0)
                                    ss_fut = (emit_scores(h, qm, kc + 2)
                                              if kc + 2 < nkc else None)
                                    ss_cur, ss_nxt = ss_nxt, ss_fut
                                    st_, sp_ = (kc == 0), (kc == nkc - 1)
                                    for hh in range(HDc // 128):
                                        mm(at[hh][:],
                                           V[kc][:, h * HDc + hh * 128:
                                                 h * HDc + (hh + 1) * 128],
                                           pt[:], start=st_, stop=sp_)
                                    mm(psRS[0:1, :],
                                       ones[:], pt[:], start=st_, stop=sp_)
                                nc.vector.reciprocal(
                                    rrb[0:1, h * NSC + qm, :], psRS[0:1, :])
                                qsl = slice(qm * SC, (qm + 1) * SC)
                                for hh in range(HDc // 128):
                                    if hh == 0:
                                        nc.scalar.copy(ATN[:, c0 + hh, qsl],
                                                       at[hh][:])
                                    else:
                                        nc.vector.tensor_copy(
                                            ATN[:, c0 + hh, qsl], at[hh][:])
                                i2 = (h * NSC + qm) % 2
                                bc_ps = bcps.tile([128, SC], fp32,
                                                  tag=f"bc{i2}", name=f"bc{i2}")
                                mm(bc_ps[:], onesr[:],
                                   rrb[0:1, h * NSC + qm, :],
                                   start=True, stop=True)
                                bcs = bcsb.tile([128, SC], bf16, tag=f"bcs{i2}",
                                                name=f"bcs{i2}")
                                nc.vector.tensor_copy(bcs[:], bc_ps[:])
                                for hh in range(HDc // 128):
                                    nc.vector.tensor_tensor(
                                        ATN[:, c0 + hh, qsl],
                                        ATN[:, c0 + hh, qsl],
                                        bcs[:], op=mybir.AluOpType.mult)

                        # ship ATN token-slices + recips to owning cores
                        for r in range(n_cores):
                            nc.scalar.dma_start(
                                a2a_in[b][r],
                                ATN[:, :, r * TPC:(r + 1) * TPC])
                        if use_collective:
                            nc.gpsimd.collective_compute(
                                "AllToAll", mybir.AluOpType.bypass,
                                replica_groups=[list(range(n_cores))],
                                ins=[a2a_in[b][:]], outs=[a2a_out[b][:]])
                        else:
                            nc.sync.dma_start(a2a_out[b][:], a2a_in[b][:])


                # ===== phase C': local out-projection over owned tokens ==
                # two half-token passes: half 0 (b=0 columns) starts right
                # after B(b1) and hides the b=1 AllToAll; half 1 follows
                with (
                    tc.tile_pool(name="wo2", bufs=6) as wo2p,
                    tc.tile_pool(name="ysbT", bufs=2) as ysbtp,
                    tc.tile_pool(name="ypsT", bufs=2, space="PSUM") as ypstp,
                ):
                    for half in range(Bc):
                        if half == 1:
                            emit_recv(1)
                        hsl = slice(half * TPC, (half + 1) * TPC)
                        for dsub in range(NDC):
                            wo2 = wo2p.tile([128, NDC, 128], bf16, tag="wo2")
                            nc.sync.dma_start(wo2[:], wo_e[dsub])
                            yT = ypstp.tile([128, TPC], fp32,
                                            tag=f"yT{dsub % 2}",
                                            name=f"yT{dsub % 2}")
                            for cg in range(NDC):
                                s, c = cg // NHC, cg % NHC
                                mm(yT[:], wo2[:, cg, :], xn[s][:, c, hsl],
                                   start=(cg == 0), stop=(cg == NDC - 1))
                            ysbT = ysbtp.tile([128, TPC], bf16, tag="ysbT")
                            if dsub % 2 == 0:
                                nc.scalar.copy(ysbT[:], yT[:])
                            else:
                                nc.vector.tensor_copy(ysbT[:], yT[:])
                            nc.gpsimd.dma_start(
                                y_e[dsub * 128:(dsub + 1) * 128, hsl],
                                ysbT[:])

    nc.compile()
    return nc


# ---------------------------------------------------------------- host prep

def _sinusoidal_np(num_pos, dim):
    inv_freq = 1.0 / (10000.0 ** (np.arange(0, dim, 2, dtype=np.float32) / dim))
    t = np.arange(num_pos, dtype=np.float32)[:, None] * inv_freq[None, :]
    return np.cos(t).astype(np.float32), np.sin(t).astype(np.float32)


def _host_arrays(hs, Wq, Wk, Wv, Wo, position_ids, cfg, n_cores):
    """Build the shared + per-core input arrays (pre-swizzled, bf16)."""
    import ml_dtypes
    bf = ml_dtypes.bfloat16
    Bc, Sc, Dc, HPCc, HDc, ROTc = (
        cfg["B"], cfg["S"], cfg["D"], cfg["HPC"], cfg["HD"], cfg["ROT"])
    HDLc = HPCc * HDc
    NSCc, NDCc, NGc, NOCc, NHCc = Sc // SC, Dc // 128, Dc // 1024, Dc // SC, HDLc // 128

    # hs_s[b, st, hf, p, j, f] = hs[b, st*SC+f, (hf*16+j)*128+p]
    hs_s = np.ascontiguousarray(
        hs.reshape(Bc, NSCc, SC, 4, NDCc // 4, 128)
        .transpose(0, 1, 3, 5, 4, 2)).astype(bf)

    cos_t, sin_t = _sinusoidal_np(max(MAX_POS, Sc), ROTc)   # [P, ROT//2]
    pos = np.asarray(position_ids).astype(np.int64)         # [B, S]
    cosg = cos_t[pos]                                       # [B, S, 32]
    sing = sin_t[pos]
    cosb = np.repeat(cosg.transpose(0, 2, 1), 2, axis=1)    # [B, 64, S]
    sinb_r = np.repeat(sing.transpose(0, 2, 1), 2, axis=1)
    sgn = np.ones((ROTc, 1), np.float32)
    sgn[0::2] = -1.0
    sinb = np.ascontiguousarray(sinb_r * sgn).astype(bf)
    cosb = np.ascontiguousarray(cosb).astype(bf)

    # transposed causal masks for diagonal 512 macro tile: masksT[k, kc, q]
    masksT = np.zeros((128, 4, SC), np.float32)
    kk = np.arange(128)[:, None]
    qq = np.arange(SC)[None, :]
    for m in range(4):
        masksT[:, m, :] = np.where(m * 128 + kk <= qq, 0.0, NEG)
    masksT = masksT.astype(bf)

    pswap = np.zeros((128, ROTc), np.float32)
    for f in range(ROTc // 2):
        pswap[2 * f + 1, 2 * f] = 1.0
        pswap[2 * f, 2 * f + 1] = 1.0
    onesc = np.ones((128, 1), np.float32).astype(bf)
    onesr = np.ones((1, 128), np.float32)

    # wo2_s[dsub, p, cg, m] = Wo[dsub*128+m, cg*128+p] (full Wo, shared)
    wo2_s = np.ascontiguousarray(
        np.asarray(Wo).reshape(NDCc, 128, NDCc, 128)
        .transpose(0, 3, 2, 1)).astype(bf)

    shared = dict(hs_s=hs_s, cosb=cosb, sinb=sinb, masksT=masksT,
                  pswap=pswap.astype(bf), onesc=onesc, onesr=onesr,
                  wo_s=wo2_s)

    def _wswz_qk(w):   # [HDLc(rows of W slice), Dc] -> [2, NG, 128, 8, HDLc//2]
        # w here is the [HDLc, Dc] row-slice of the full weight; stationary
        # layout wq_s[hf, g, p, j, f] = w[hf*256+f, (g*8+j)*128+p]
        return np.ascontiguousarray(
            w.reshape(2, HDLc // 2, NGc, 8, 128)
            .transpose(0, 2, 4, 3, 1)).astype(bf)

    def _wswz_v(w):    # -> [NG, 128, 8, HDLc];  wv_s[g,p,j,f] = w[f,(g*8+j)*128+p]
        return np.ascontiguousarray(
            w.reshape(HDLc, NGc, 8, 128).transpose(1, 3, 2, 0)).astype(bf)

    per_core = []
    for c in range(n_cores):
        csl = slice(c * HDLc, (c + 1) * HDLc)
        per_core.append(dict(
            wq_s=_wswz_qk(np.asarray(Wq)[csl, :]),
            wk_s=_wswz_qk(np.asarray(Wk)[csl, :]),
            wv_s=_wswz_v(np.asarray(Wv)[csl, :]),
            **shared,
        ))
    return per_core


def _numpy_reference(hidden_states, Wq, Wk, Wv, Wo, layer_past_k, layer_past_v,
                     attention_mask, position_ids, new_key_loc, new_value_loc,
                     valid_key_indices, valid_value_indices, bucket_size):
    """Slow but general fallback (mirrors reference.py in numpy fp32)."""
    hs = np.asarray(hidden_states, np.float32)
    Bc, Sc, Dc = hs.shape
    q = (hs @ np.asarray(Wq).T).reshape(Bc, Sc, NH, HD)
    k = (hs @ np.asarray(Wk).T).reshape(Bc, Sc, NH, HD)
    v = (hs @ np.asarray(Wv).T).reshape(Bc, Sc, NH, HD)

    cos_t, sin_t = _sinusoidal_np(MAX_POS, ROT)
    pos = np.asarray(position_ids).astype(np.int64)
    c_ = cos_t[pos][:, :, None, :]      # [B,S,1,32]
    s_ = sin_t[pos][:, :, None, :]

    def rot(x):
        xr = x[..., :ROT].reshape(Bc, Sc, NH, ROT // 2, 2)
        x0, x1 = xr[..., 0], xr[..., 1]
        o0 = c_ * x0 - s_ * x1
        o1 = s_ * x0 + c_ * x1
        out = np.stack([o0, o1], axis=-1).reshape(Bc, Sc, NH, ROT)
        return np.concatenate([out, x[..., ROT:]], axis=-1)

    q, k = rot(q), rot(k)
    nk = np.asarray(layer_past_k, np.float32).copy()
    nv = np.asarray(layer_past_v, np.float32).copy()
    nk[np.asarray(new_key_loc)] = k.reshape(Bc * Sc, 1, NH, HD)
    nv[np.asarray(new_value_loc)] = v.reshape(Bc * Sc, 1, NH, HD)
    kg = nk[np.asarray(valid_key_indices)].reshape(
        Bc, bucket_size, NH, HD).transpose(0, 2, 1, 3)
    vg = nv[np.asarray(valid_value_indices)].reshape(
        Bc, bucket_size, NH, HD).transpose(0, 2, 1, 3)
    qh = q.transpose(0, 2, 1, 3)
    scores = np.einsum("bhqd,bhkd->bhqk", qh, kg)
    causal = np.tril(np.ones((MAX_POS, MAX_POS), bool))[
        bucket_size - Sc:bucket_size, :bucket_size]
    scores = np.where(causal, scores, np.float32(np.finfo(np.float32).min))
    scores = scores / np.float32(np.sqrt(HD)) + np.asarray(attention_mask, np.float32)
    scores = scores - scores.max(-1, keepdims=True)
    p = np.exp(scores)
    p = p / p.sum(-1, keepdims=True)
    attn = np.einsum("bhqk,bhkd->bhqd", p, vg)
    attn = attn.transpose(0, 2, 1, 3).reshape(Bc, Sc, Dc)
    return (attn @ np.asarray(Wo).T).astype(np.float32)


def _fast_path_ok(layer_past_k, layer_past_v, attention_mask, new_key_loc,
                  new_value_loc, valid_key_indices, valid_value_indices,
                  bucket_size, hs_shape):
    Bc, Sc, Dc = hs_shape
    if (Bc, Sc, Dc) != (B, S, D) or int(bucket_size) != S:
        return False
    ar = np.arange(Bc * Sc)
    for idx in (new_key_loc, new_value_loc, valid_key_indices, valid_value_indices):
        a = np.asarray(idx)
        if a.shape != (Bc * Sc,) or not np.array_equal(a, ar):
            return False
    if np.any(np.asarray(attention_mask) != 0):
        return False
    return True


_NC_CACHE = {}


def _get_nc(use_collective=True):
    key = ("v2", use_collective)
    if key not in _NC_CACHE:
        _NC_CACHE[key] = build_nc(_cfg_full(), use_collective=use_collective,
                                  n_cores=N_CORES)
    return _NC_CACHE[key]


def _assemble(outs, use_collective):
    # core r returns y^T [D, B*TPC]; its tokens are b*S + r*TPC + i
    TPC = S // len(outs)
    stk = np.stack([np.asarray(o, np.float32) for o in outs])  # [R, D, B*TPC]
    R = stk.shape[0]
    y = stk.reshape(R, D, B, TPC).transpose(2, 0, 3, 1).reshape(B * S, D)
    return y


def kernel(**inputs):
    hs = np.asarray(inputs["hidden_states"], np.float32)
    fast = _fast_path_ok(
        inputs["layer_past_k"], inputs["layer_past_v"], inputs["attention_mask"],
        inputs["new_key_loc"], inputs["new_value_loc"],
        inputs["valid_key_indices"], inputs["valid_value_indices"],
        inputs["bucket_size"], hs.shape)
    if not fast:
        return _numpy_reference(**inputs)

    from concourse.bass_utils import run_bass_kernel_spmd

    use_collective = os.environ.get("KERNEL_NO_COLLECTIVE", "") != "1"
    nc = _get_nc(use_collective)
    in_maps = _host_arrays(
        hs, np.asarray(inputs["Wq"], np.float32),
        np.asarray(inputs["Wk"], np.float32),
        np.asarray(inputs["Wv"], np.float32),
        np.asarray(inputs["Wo"], np.float32),
        inputs["position_ids"], _cfg_full(), N_CORES)
    res = run_bass_kernel_spmd(nc, in_maps, list(range(N_CORES)))
    outs = [res.results[c]["y"] for c in range(N_CORES)]
    y = _assemble(outs, use_collective)
    return y.reshape(B, S, D).astype(np.float32)
